# revision 22
# baseline (speedup 1.0000x reference)
"""CapsuleLayer dynamic-routing kernel for 8 Trainium2 NeuronCores.

Math (reference):
    u_hat[b,n,j,d] = sum_i W[n,j,d,i] * x[b,j,i]
    b = 0; for r in 0..2:
        c = softmax_n(b); s[b,n,d] = sum_j c*u_hat; v = squash_d(s)
        if r < 2: b += sum_d v*u_hat
    return v  [B, N, D]

Sharding: J (input capsules, 2048) split 8 ways -> Jc=256 per core.
Softmax over n is local; only s needs a 256 KiB AllReduce per iteration.

Per-core design (v3) — everything is (d, n)-major in the free dim:
  r0: c uniform -> s0 = (1/N) sum_{j,i} x*W, via K=128 matmuls over
      (j8,i) chunks; W layout-A [(j8,i), (d,n)] spans 128 partitions.
  r1/r2: j groups of 4 on DIAGONAL PE tiles (row band 32a = j=4g+a,
      operands at partitions 32a..32a+16; W tiles span 128 partitions,
      2 MiB DMAs). Per group:
        u in PSUM -> scalar evac bf16 [128, (d,n)]
        tl = u * v_rep           (DVE TT, dense, 2x)
        logits = sum_d tl        (5-level dense TT pyramid over outer d)
      Per supergroup of 4 groups: e_raw = exp(logits) batched; per
      group zsum/recip/log -> e2 = exp(logits - lnZ) on scalar (bias is
      per-partition), so t2 = e2 * u with an OUTER-dim broadcast of e2
      (dense inner runs -> fast on DVE/GpSimd; no stride-0 inner dims).
      s accumulated over all groups in one PSUM bank via ones4 matmuls.
  AllReduce s in fp32 via DRAM bounce; squash on the gathered [B,(d,n)]
  form (d-sum = dense pyramid); output transposed to [B,N,D] on host.
"""

import functools
import numpy as np

B, J, I = 32, 2048, 16
N, D = 64, 32
NCORES = 8
JC = J // NCORES          # 256 j per core
GRP = 4                   # j's per group (PE diagonal bands)
NG = JC // GRP            # 64 groups
SG = 4                    # groups per supergroup
NSG = NG // SG            # 16 supergroups
NCH = JC // 8             # 32 K=128 chunks for r0
ND = N * D                # 2048
HALF = ND // 2            # 1024
ROUTINGS = 3
EPS = 1e-7

GS_T2 = 3                 # how many of each supergroup's 4 t2 ops go to gpsimd


@functools.lru_cache(maxsize=1)
def _build():
    import concourse.bass as bass
    import concourse.mybir as mybir
    import concourse.bacc as bacc
    import concourse.tile as tile

    f32 = mybir.dt.float32
    bf16 = mybir.dt.bfloat16
    MUL = mybir.AluOpType.mult
    ADD = mybir.AluOpType.add
    AX = mybir.AxisListType.X
    AF = mybir.ActivationFunctionType

    nc = bacc.Bacc("TRN2", target_bir_lowering=False, debug=False,
                   num_devices=NCORES)

    xa_d = nc.dram_tensor("xa", [128, NCH, B], bf16, kind="ExternalInput")
    wa_d = nc.dram_tensor("wa", [128, NCH, ND], bf16, kind="ExternalInput")
    xb_d = nc.dram_tensor("xb", [128, NG, B], bf16, kind="ExternalInput")
    wb_d = nc.dram_tensor("wb", [NSG, 128, SG * ND], bf16, kind="ExternalInput")
    ones_d = nc.dram_tensor("ones4", [128, B], bf16, kind="ExternalInput")
    v_d = nc.dram_tensor("v", [B, ND], f32, kind="ExternalOutput")

    with tile.TileContext(nc) as tc:
        with (
            tc.tile_pool(name="persist", bufs=1) as pp,
            tc.tile_pool(name="wstream", bufs=2) as wp,
            tc.tile_pool(name="usb", bufs=10) as up,
            tc.tile_pool(name="tl", bufs=3) as tp,
            tc.tile_pool(name="pyr", bufs=2) as pyp,
            tc.tile_pool(name="t2", bufs=3) as t2p,
            tc.tile_pool(name="small", bufs=2) as sm,
            tc.tile_pool(name="soft", bufs=4) as sfp,
            tc.tile_pool(name="ups", bufs=3, space="PSUM") as ups_pool,
            tc.tile_pool(name="sps", bufs=1, space="PSUM") as sps_pool,
            tc.tile_pool(name="dram", bufs=1, space="DRAM") as dr,
        ):
            xa = pp.tile([128, NCH, B], bf16)
            nc.sync.dma_start(xa[:], xa_d[:])
            xb = pp.tile([128, NG, B], bf16)
            nc.sync.dma_start(xb[:], xb_d[:])
            ones4 = pp.tile([128, B], bf16)
            nc.sync.dma_start(ones4[:], ones_d[:])

            logits = pp.tile([128, NG, N], bf16)
            v_rep = pp.tile([128, ND], bf16)
            v_small = pp.tile([B, ND], bf16)
            s_full = pp.tile([B, ND], f32)
            v_sb = pp.tile([B, ND], f32)
            s0 = pp.tile([B, ND], f32)
            s_evac = pp.tile([128, 512], f32)

            cc_in = dr.tile([128, 512], f32)
            cc_out = dr.tile([128, 512], f32)

            def cc_and_squash(r):
                """AllReduce cc_in -> cc_out, gather to [B,(d,n)], squash
                with a dense outer-d pyramid, write v_rep (r<2) / v_sb."""
                nc.gpsimd.collective_compute(
                    "AllReduce", ADD,
                    replica_groups=[list(range(NCORES))],
                    ins=[cc_in[:].opt()], outs=[cc_out[:].opt()],
                )
                for q in range(4):
                    nc.sync.dma_start(
                        s_full[:, q * 512:(q + 1) * 512],
                        cc_out[32 * q:32 * q + 32, :])

                s3 = s_full[:].rearrange("p (d n) -> p d n", n=N)
                sq = sm.tile([B, D, N], f32, name="sq", tag="sq")
                nc.vector.tensor_tensor(sq[:], s3, s3, op=MUL)
                h = D
                cur = sq
                while h > 1:
                    h //= 2
                    nxt = sm.tile([B, h, N], f32, name=f"sp{h}", tag=f"sp{h}")
                    nc.vector.tensor_tensor(
                        nxt[:], cur[:, 0:h, :], cur[:, h:2 * h, :], op=ADD)
                    cur = nxt
                ns2 = cur  # [B, 1, N]
                ns2f = ns2[:].rearrange("p a n -> p (a n)")
                onep = sm.tile([B, N], f32, name="onep", tag="onep")
                nc.vector.tensor_scalar_add(onep[:], ns2f, 1.0)
                rt = sm.tile([B, N], f32, name="rt", tag="rt")
                eps_t = sm.tile([B, 1], f32, name="eps", tag="eps")
                nc.vector.memset(eps_t[:], EPS)
                nc.scalar.activation(rt[:], ns2f, AF.Sqrt, bias=eps_t[:])
                den = sm.tile([B, N], f32, name="den", tag="den")
                nc.vector.tensor_tensor(den[:], onep[:], rt[:], op=MUL)
                dinv = sm.tile([B, N], f32, name="dinv", tag="dinv")
                nc.vector.reciprocal(dinv[:], den[:])
                scl = sm.tile([B, N], f32, name="scl", tag="scl")
                nc.vector.tensor_tensor(scl[:], ns2f, dinv[:], op=MUL)

                if r < ROUTINGS - 1:
                    nc.vector.tensor_tensor(
                        v_small[:].rearrange("p (d n) -> p d n", n=N), s3,
                        scl[:, None, :].broadcast_to([B, D, N]),
                        op=MUL)
                    for rr in range(GRP):
                        nc.sync.dma_start(
                            v_rep[32 * rr:32 * rr + 32, :], v_small[:])
                else:
                    nc.vector.tensor_tensor(
                        v_sb[:].rearrange("p (d n) -> p d n", n=N), s3,
                        scl[:, None, :].broadcast_to([B, D, N]),
                        op=MUL)

            # ---------------- r0: uniform c ----------------
            acc = [ups_pool.tile([128, HALF], f32, name="u_ps", tag="ups")
                   for _h in range(2)]
            CHB = 4
            for cb in range(NCH // CHB):
                wt = wp.tile([128, CHB * ND], bf16, name="w_t", tag="wst")
                nc.sync.dma_start(
                    wt[:], wa_d[:, cb * CHB:(cb + 1) * CHB, :])
                wt = wt[:].rearrange("p (c f) -> p c f", c=CHB)
                for cc_ in range(CHB):
                    ch = cb * CHB + cc_
                    for h in range(2):
                        for q in range(2):
                            nc.tensor.matmul(
                                acc[h][0:B, q * 512:(q + 1) * 512],
                                xa[:, ch, :],
                                wt[:, cc_, h * HALF + q * 512:
                                    h * HALF + (q + 1) * 512],
                                start=(ch == 0), stop=(ch == NCH - 1),
                                skip_group_check=True,
                            )
            for h in range(2):
                nc.scalar.activation(
                    s0[:, h * HALF:(h + 1) * HALF], acc[h][0:B, :], AF.Copy)
            for q in range(4):
                nc.sync.dma_start(
                    cc_in[32 * q:32 * q + 32, :],
                    s0[:, q * 512:(q + 1) * 512])
            cc_and_squash(0)

            # ---------------- r1, r2 ----------------
            for r in range(1, ROUTINGS):
                s_ps = sps_pool.tile([128, 512], f32)

                state = []

                def u_phase(sg):
                    wt = wp.tile([128, SG * ND], bf16, name="w_t", tag="wst")
                    nc.sync.dma_start(wt[:], wb_d[sg, :, :])
                    wtv = wt[:].rearrange("p (g f) -> p g f", g=SG)
                    u_sbs = []
                    for gi in range(SG):
                        g = sg * SG + gi
                        u_sb = up.tile([128, ND], bf16, name="u_sb", tag="usb")
                        for h in range(2):
                            u_ps = ups_pool.tile([128, HALF], f32,
                                                 name="u_ps", tag="ups")
                            for a in range(GRP):
                                for q in range(2):
                                    nc.tensor.matmul(
                                        u_ps[32 * a:32 * a + 32,
                                             q * 512:(q + 1) * 512],
                                        xb[32 * a:32 * a + 16, g, :],
                                        wtv[32 * a:32 * a + 16, gi,
                                            h * HALF + q * 512:
                                            h * HALF + (q + 1) * 512],
                                        start=True, stop=True,
                                        tile_position=(32 * a, 32 * a),
                                        skip_group_check=True,
                                    )
                            nc.scalar.activation(
                                u_sb[:, h * HALF:(h + 1) * HALF],
                                u_ps[:], AF.Copy)
                        # tl = u * v_rep  (flat 2D, bf16, 2x mode)
                        tl = tp.tile([128, ND], bf16, name="tl", tag="tl")
                        nc.vector.tensor_tensor(tl[:], u_sb[:], v_rep[:],
                                                op=MUL)
                        # sum over outer d: dense 5-level pyramid
                        with nc.allow_low_precision("bf16 logits pyramid"):
                            cur = tl[:].rearrange("p (d n) -> p d n", n=N)
                            h2 = D
                            while h2 > 2:
                                h2 //= 2
                                nxt = pyp.tile([128, h2, N], bf16,
                                               name=f"py{h2}", tag=f"py{h2}")
                                nc.vector.tensor_tensor(
                                    nxt[:], cur[:, 0:h2, :],
                                    cur[:, h2:2 * h2, :], op=ADD)
                                cur = nxt[:]
                            if r == 1:
                                nc.vector.tensor_tensor(
                                    logits[:, g, :], cur[:, 0, :],
                                    cur[:, 1, :], op=ADD)
                            else:
                                dtmp = sm.tile([128, N], bf16, name="dtmp",
                                               tag="dtmp")
                                nc.vector.tensor_tensor(
                                    dtmp[:], cur[:, 0, :], cur[:, 1, :],
                                    op=ADD)
                                nc.vector.tensor_add(
                                    logits[:, g, :], logits[:, g, :],
                                    dtmp[:])
                        u_sbs.append(u_sb)
                    return u_sbs

                def s_phase(sg, u_sbs):
                    sl = slice(sg * SG, (sg + 1) * SG)
                    e_raw = sfp.tile([128, SG, N], bf16, name="eraw",
                                     tag="eraw")
                    nc.scalar.activation(e_raw[:], logits[:, sl, :], AF.Exp)
                    for gi in range(SG):
                        g = sg * SG + gi
                        zsum = sfp.tile([128, 1], f32, name="zs", tag="zs")
                        nc.vector.tensor_reduce(
                            zsum[:], e_raw[:, gi, :], axis=AX, op=ADD)
                        zrec = sfp.tile([128, 1], f32, name="zr", tag="zr")
                        nc.vector.reciprocal(zrec[:], zsum[:])
                        nlz = sfp.tile([128, 1], f32, name="nlz", tag="nlz")
                        nc.scalar.activation(nlz[:], zrec[:], AF.Ln)
                        e2 = sfp.tile([128, N], bf16, name="e2", tag="e2")
                        nc.scalar.activation(e2[:], logits[:, g, :], AF.Exp,
                                             bias=nlz[:])
                        t2 = t2p.tile([128, ND], bf16, name="t2", tag="t2")
                        eng = nc.gpsimd if gi < GS_T2 else nc.vector
                        eng.tensor_tensor(
                            t2[:].rearrange("p (d n) -> p d n", n=N),
                            u_sbs[gi][:].rearrange("p (d n) -> p d n", n=N),
                            e2[:, None, :].broadcast_to([128, D, N]),
                            op=MUL)
                        for q in range(4):
                            nc.tensor.matmul(
                                s_ps[32 * q:32 * q + 32, :],
                                ones4[:],
                                t2[:, q * 512:(q + 1) * 512],
                                start=(g == 0), stop=(g == NG - 1),
                                tile_position=(0, 32 * q),
                                skip_group_check=True,
                            )

                for sg in range(NSG):
                    u_sbs = u_phase(sg)
                    if state:
                        s_phase(*state.pop(0))
                    state.append((sg, u_sbs))
                while state:
                    s_phase(*state.pop(0))

                nc.vector.tensor_copy(s_evac[:], s_ps[:])
                nc.sync.dma_start(cc_in[:], s_evac[:])
                cc_and_squash(r)

            nc.sync.dma_start(v_d[:], v_sb[:])

    nc.compile()
    return nc


def prepare_inputs(x: np.ndarray, W: np.ndarray):
    """Full inputs -> per-core input maps (host-side reshuffles).
    All free dims are (d, n)-major."""
    import ml_dtypes
    bf = ml_dtypes.bfloat16

    ones4 = np.tile(np.eye(B, dtype=np.float32), (GRP, 1)).astype(bf)

    in_maps = []
    for k in range(NCORES):
        jlo, jhi = k * JC, (k + 1) * JC
        Wc = W[:, jlo:jhi]                       # [N, JC, D, I]
        xc = x[:, jlo:jhi]                       # [B, JC, I]
        arrw = np.ascontiguousarray(
            Wc.transpose(1, 3, 2, 0)).reshape(JC, I, ND)   # [j, i, (d,n)]
        arrx = np.ascontiguousarray(xc.transpose(1, 2, 0))  # [j, i, b]

        wa = arrw.reshape(NCH, 8, I, ND).transpose(1, 2, 0, 3) \
            .reshape(128, NCH, ND)
        xa = (arrx / N).reshape(NCH, 8, I, B).transpose(1, 2, 0, 3) \
            .reshape(128, NCH, B)

        wb4 = arrw.reshape(NSG, SG, GRP, I, ND)   # [sg, gi, a, i, nd]
        wbp = np.zeros((NSG, GRP, 32, SG, ND), dtype=np.float32)
        wbp[:, :, :I] = wb4.transpose(0, 2, 3, 1, 4)  # [sg, a, i, gi, nd]
        wb = wbp.reshape(NSG, 128, SG * ND)

        xb4 = arrx.reshape(NG, GRP, I, B)         # [g, a, i, b]
        xbp = np.zeros((GRP, 32, NG, B), dtype=np.float32)
        xbp[:, :I] = xb4.transpose(1, 2, 0, 3)    # [a, i, g, b]
        xb = xbp.reshape(128, NG, B)

        in_maps.append({
            "xa": np.ascontiguousarray(xa).astype(bf),
            "wa": np.ascontiguousarray(wa).astype(bf),
            "xb": np.ascontiguousarray(xb).astype(bf),
            "wb": np.ascontiguousarray(wb).astype(bf),
            "ones4": ones4,
        })
    return in_maps


def kernel(x: np.ndarray, W: np.ndarray) -> np.ndarray:
    from concourse.bass_utils import run_bass_kernel_spmd

    nc = _build()
    in_maps = prepare_inputs(x, W)
    res = run_bass_kernel_spmd(nc, in_maps, list(range(NCORES)))
    v = np.asarray(res.results[0]["v"], dtype=np.float32)
    # device layout is [B, (d, n)] -> [B, N, D]
    return np.ascontiguousarray(
        v.reshape(B, D, N).transpose(0, 2, 1))


if __name__ == "__main__":
    rng = np.random.default_rng(0)
    x = rng.normal(size=(B, J, I)).astype(np.float32)
    W = rng.normal(size=(N, J, D, I)).astype(np.float32) * 0.05
    v = kernel(x, W)
    print(v.shape, v.dtype, np.abs(v).max())


# revision 32
# speedup vs baseline: 1.2152x; 1.2152x over previous
"""CapsuleLayer dynamic-routing kernel for 8 Trainium2 NeuronCores.

Math (reference):
    u_hat[b,n,j,d] = sum_i W[n,j,d,i] * x[b,j,i]
    b = 0; for r in 0..2:
        c = softmax_n(b); s[b,n,d] = sum_j c*u_hat; v = squash_d(s)
        if r < 2: b += sum_d v*u_hat
    return v  [B, N, D]

Sharding: J (input capsules, 2048) split 8 ways -> Jc=256 per core.
Softmax over n is local; only s needs a 256 KiB AllReduce per iteration.

Per-core design (v3) — everything is (d, n)-major in the free dim:
  r0: c uniform -> s0 = (1/N) sum_{j,i} x*W, via K=128 matmuls over
      (j8,i) chunks; W layout-A [(j8,i), (d,n)] spans 128 partitions.
  r1/r2: j groups of 4 on DIAGONAL PE tiles (row band 32a = j=4g+a,
      operands at partitions 32a..32a+16; W tiles span 128 partitions,
      2 MiB DMAs). Per group:
        u in PSUM -> scalar evac bf16 [128, (d,n)]
        tl = u * v_rep           (DVE TT, dense, 2x)
        logits = sum_d tl        (5-level dense TT pyramid over outer d)
      Per supergroup of 4 groups: e_raw = exp(logits) batched; per
      group zsum/recip/log -> e2 = exp(logits - lnZ) on scalar (bias is
      per-partition), so t2 = e2 * u with an OUTER-dim broadcast of e2
      (dense inner runs -> fast on DVE/GpSimd; no stride-0 inner dims).
      s accumulated over all groups in one PSUM bank via ones4 matmuls.
  AllReduce s in fp32 via DRAM bounce; squash on the gathered [B,(d,n)]
  form (d-sum = dense pyramid); output transposed to [B,N,D] on host.
"""

import functools
import numpy as np

B, J, I = 32, 2048, 16
N, D = 64, 32
NCORES = 8
JC = J // NCORES          # 256 j per core
GRP = 4                   # j's per group (PE diagonal bands)
NG = JC // GRP            # 64 groups
SG = 4                    # groups per supergroup
NSG = NG // SG            # 16 supergroups
NCH = JC // 8             # 32 K=128 chunks for r0
ND = N * D                # 2048
HALF = ND // 2            # 1024
ROUTINGS = 3
EPS = 1e-7

GS_T2 = 3                 # how many of each supergroup's 4 t2 ops go to gpsimd


@functools.lru_cache(maxsize=1)
def _build():
    import concourse.bass as bass
    import concourse.mybir as mybir
    import concourse.bacc as bacc
    import concourse.tile as tile

    f32 = mybir.dt.float32
    bf16 = mybir.dt.bfloat16
    MUL = mybir.AluOpType.mult
    ADD = mybir.AluOpType.add
    AX = mybir.AxisListType.X
    AF = mybir.ActivationFunctionType

    nc = bacc.Bacc("TRN2", target_bir_lowering=False, debug=False,
                   num_devices=NCORES)

    xa_d = nc.dram_tensor("xa", [128, NCH, B], bf16, kind="ExternalInput")
    wa_d = nc.dram_tensor("wa", [128, NCH, ND], bf16, kind="ExternalInput")
    xb_d = nc.dram_tensor("xb", [128, NG, B], bf16, kind="ExternalInput")
    wb_d = nc.dram_tensor("wb", [NSG, 128, SG * ND], bf16, kind="ExternalInput")
    ones_d = nc.dram_tensor("ones4", [128, B], bf16, kind="ExternalInput")
    v_d = nc.dram_tensor("v", [B, ND], f32, kind="ExternalOutput")

    with tile.TileContext(nc) as tc:
        with (
            tc.tile_pool(name="persist", bufs=1) as pp,
            tc.tile_pool(name="wstream", bufs=2) as wp,
            tc.tile_pool(name="usb", bufs=10) as up,
            tc.tile_pool(name="tl", bufs=3) as tp,
            tc.tile_pool(name="pyr", bufs=2) as pyp,
            tc.tile_pool(name="t2", bufs=2) as t2p,
            tc.tile_pool(name="small", bufs=1) as sm,
            tc.tile_pool(name="soft", bufs=4) as sfp,
            tc.tile_pool(name="ups", bufs=3, space="PSUM") as ups_pool,
            tc.tile_pool(name="sps", bufs=1, space="PSUM") as sps_pool,
            tc.tile_pool(name="dram", bufs=1, space="DRAM") as dr,
        ):
            xa = pp.tile([128, NCH, B], bf16)
            nc.sync.dma_start(xa[:], xa_d[:])
            xb = pp.tile([128, NG, B], bf16)
            nc.sync.dma_start(xb[:], xb_d[:])
            ones4 = pp.tile([128, B], bf16)
            nc.sync.dma_start(ones4[:], ones_d[:])

            logits = pp.tile([128, NG, N], bf16)
            v_rep = pp.tile([128, ND], bf16)
            v_small = pp.tile([B, ND], bf16)
            s_full = pp.tile([B, ND], f32)
            v_sb = pp.tile([B, ND], f32)
            s0 = pp.tile([B, ND], f32)
            s_evac = pp.tile([128, 512], f32)
            sqt = pp.tile([B, ND], f32)
            sclFt = pp.tile([B, ND], f32)

            cc_in = dr.tile([128, 512], f32)
            cc_out = dr.tile([128, 512], f32)

            def cc_and_squash(r):
                """AllReduce cc_in -> cc_out, gather to [B,(d,n)], squash
                with a dense outer-d pyramid, write v_rep (r<2) / v_sb."""
                nc.gpsimd.collective_compute(
                    "AllReduce", ADD,
                    replica_groups=[list(range(NCORES))],
                    ins=[cc_in[:].opt()], outs=[cc_out[:].opt()],
                )
                for q in range(4):
                    nc.sync.dma_start(
                        s_full[:, q * 512:(q + 1) * 512],
                        cc_out[32 * q:32 * q + 32, :])

                s3 = s_full[:].rearrange("p (d n) -> p d n", n=N)
                sq = sqt[:].rearrange("p (d n) -> p d n", n=N)
                nc.vector.tensor_tensor(sq, s3, s3, op=MUL)
                h = D
                cur = sq
                while h > 1:
                    h //= 2
                    nxt = sm.tile([B, h, N], f32, name=f"sp{h}", tag=f"sp{h}")
                    nc.vector.tensor_tensor(
                        nxt[:], cur[:, 0:h, :], cur[:, h:2 * h, :], op=ADD)
                    cur = nxt[:]
                ns2f = cur.rearrange("p a n -> p (a n)")
                onep = sm.tile([B, N], f32, name="onep", tag="onep")
                nc.vector.tensor_scalar_add(onep[:], ns2f, 1.0)
                rt = sm.tile([B, N], f32, name="rt", tag="rt")
                eps_t = sm.tile([B, 1], f32, name="eps", tag="eps")
                nc.vector.memset(eps_t[:], EPS)
                nc.scalar.activation(rt[:], ns2f, AF.Sqrt, bias=eps_t[:])
                den = sm.tile([B, N], f32, name="den", tag="den")
                nc.vector.tensor_tensor(den[:], onep[:], rt[:], op=MUL)
                dinv = sm.tile([B, N], f32, name="dinv", tag="dinv")
                nc.vector.reciprocal(dinv[:], den[:])
                scl = sm.tile([B, N], f32, name="scl", tag="scl")
                nc.vector.tensor_tensor(scl[:], ns2f, dinv[:], op=MUL)
                sclF = sclFt[:].rearrange("p (d n) -> p d n", n=N)
                nc.scalar.activation(
                    sclF, scl[:, None, :].broadcast_to([B, D, N]),
                    AF.Copy)

                if r < ROUTINGS - 1:
                    nc.vector.tensor_tensor(
                        v_small[:].rearrange("p (d n) -> p d n", n=N), s3,
                        sclF, op=MUL)
                    for rr in range(GRP):
                        nc.sync.dma_start(
                            v_rep[32 * rr:32 * rr + 32, :], v_small[:])
                else:
                    nc.vector.tensor_tensor(
                        v_sb[:].rearrange("p (d n) -> p d n", n=N), s3,
                        sclF, op=MUL)

            # ---------------- r0: uniform c ----------------
            acc = [ups_pool.tile([128, HALF], f32, name="u_ps", tag="ups")
                   for _h in range(2)]
            CHB = 4
            for cb in range(NCH // CHB):
                wt = wp.tile([128, CHB * ND], bf16, name="w_t", tag="wst")
                nc.sync.dma_start(
                    wt[:], wa_d[:, cb * CHB:(cb + 1) * CHB, :])
                wt = wt[:].rearrange("p (c f) -> p c f", c=CHB)
                for cc_ in range(CHB):
                    ch = cb * CHB + cc_
                    for h in range(2):
                        for q in range(2):
                            nc.tensor.matmul(
                                acc[h][0:B, q * 512:(q + 1) * 512],
                                xa[:, ch, :],
                                wt[:, cc_, h * HALF + q * 512:
                                    h * HALF + (q + 1) * 512],
                                start=(ch == 0), stop=(ch == NCH - 1),
                                skip_group_check=True,
                            )
            for h in range(2):
                nc.scalar.activation(
                    s0[:, h * HALF:(h + 1) * HALF], acc[h][0:B, :], AF.Copy)
            for q in range(4):
                nc.sync.dma_start(
                    cc_in[32 * q:32 * q + 32, :],
                    s0[:, q * 512:(q + 1) * 512])
            cc_and_squash(0)

            # ---------------- r1, r2 ----------------
            for r in range(1, ROUTINGS):
                s_ps = sps_pool.tile([128, 512], f32)

                state = []

                def u_phase(sg):
                    wt = wp.tile([128, SG * ND], bf16, name="w_t", tag="wst")
                    nc.sync.dma_start(wt[:], wb_d[sg, :, :])
                    wtv = wt[:].rearrange("p (g f) -> p g f", g=SG)
                    u_sbs = []
                    sgP = pyp.tile([128, SG, 8, N], bf16, name="sgP",
                                   tag="sgP")
                    for gi in range(SG):
                        g = sg * SG + gi
                        u_sb = up.tile([128, ND], bf16, name="u_sb", tag="usb")
                        for h in range(2):
                            u_ps = ups_pool.tile([128, HALF], f32,
                                                 name="u_ps", tag="ups")
                            for a in range(GRP):
                                for q in range(2):
                                    nc.tensor.matmul(
                                        u_ps[32 * a:32 * a + 32,
                                             q * 512:(q + 1) * 512],
                                        xb[32 * a:32 * a + 16, g, :],
                                        wtv[32 * a:32 * a + 16, gi,
                                            h * HALF + q * 512:
                                            h * HALF + (q + 1) * 512],
                                        start=True, stop=True,
                                        tile_position=(32 * a, 32 * a),
                                        skip_group_check=True,
                                    )
                            nc.scalar.activation(
                                u_sb[:, h * HALF:(h + 1) * HALF],
                                u_ps[:], AF.Copy)
                        # tl = u * v_rep  (flat 2D, bf16, 2x mode)
                        tl = tp.tile([128, ND], bf16, name="tl", tag="tl")
                        nc.vector.tensor_tensor(tl[:], u_sb[:], v_rep[:],
                                                op=MUL)
                        # sum over outer d: L1+L2 per group, tail batched
                        with nc.allow_low_precision("bf16 logits pyramid"):
                            tl3 = tl[:].rearrange("p (d n) -> p d n", n=N)
                            p16 = pyp.tile([128, 16, N], bf16, name="p16",
                                           tag="p16")
                            nc.vector.tensor_tensor(
                                p16[:], tl3[:, 0:16, :], tl3[:, 16:32, :],
                                op=ADD)
                            nc.vector.tensor_tensor(
                                sgP[:, gi], p16[:, 0:8, :], p16[:, 8:16, :],
                                op=ADD)
                        u_sbs.append(u_sb)
                    # batched pyramid tail over the supergroup
                    sl = slice(sg * SG, (sg + 1) * SG)
                    with nc.allow_low_precision("bf16 logits pyramid"):
                        t4 = pyp.tile([128, SG, 4, N], bf16, name="t4",
                                      tag="t4")
                        nc.vector.tensor_tensor(
                            t4[:], sgP[:, :, 0:4, :], sgP[:, :, 4:8, :],
                            op=ADD)
                        t2_ = pyp.tile([128, SG, 2, N], bf16, name="t2_",
                                       tag="t2_")
                        nc.vector.tensor_tensor(
                            t2_[:], t4[:, :, 0:2, :], t4[:, :, 2:4, :],
                            op=ADD)
                        if r == 1:
                            nc.vector.tensor_tensor(
                                logits[:, sl, :], t2_[:, :, 0, :],
                                t2_[:, :, 1, :], op=ADD)
                        else:
                            dsg = pyp.tile([128, SG, N], bf16, name="dsg",
                                           tag="dsg")
                            nc.vector.tensor_tensor(
                                dsg[:], t2_[:, :, 0, :], t2_[:, :, 1, :],
                                op=ADD)
                            nc.vector.tensor_add(
                                logits[:, sl, :], logits[:, sl, :], dsg[:])
                    return u_sbs

                def s_phase(sg, u_sbs):
                    sl = slice(sg * SG, (sg + 1) * SG)
                    e_raw = sfp.tile([128, SG, N], bf16, name="eraw",
                                     tag="eraw")
                    nc.scalar.activation(e_raw[:], logits[:, sl, :], AF.Exp)
                    for gi in range(SG):
                        g = sg * SG + gi
                        zsum = sfp.tile([128, 1], f32, name="zs", tag="zs")
                        nc.vector.tensor_reduce(
                            zsum[:], e_raw[:, gi, :], axis=AX, op=ADD)
                        zrec = sfp.tile([128, 1], f32, name="zr", tag="zr")
                        nc.vector.reciprocal(zrec[:], zsum[:])
                        # w4 = ones4 * zinv (dense; zinv expanded on scalar)
                        zinvB = sfp.tile([128, B], bf16, name="zb", tag="zb")
                        nc.scalar.activation(
                            zinvB[:], zrec[:].broadcast_to([128, B]),
                            AF.Copy)
                        w4 = sfp.tile([128, B], bf16, name="w4", tag="w4")
                        nc.vector.tensor_tensor(w4[:], ones4[:], zinvB[:],
                                                op=MUL)
                        t2 = t2p.tile([128, ND], bf16, name="t2", tag="t2")
                        if gi < 2:
                            # gpsimd fused broadcast multiply
                            nc.gpsimd.tensor_tensor(
                                t2[:].rearrange("p (d n) -> p d n", n=N),
                                u_sbs[gi][:].rearrange("p (d n) -> p d n",
                                                       n=N),
                                e_raw[:, gi, None, :]
                                .broadcast_to([128, D, N]),
                                op=MUL)
                        else:
                            # expand e to dense, then dense DVE multiply
                            eF = t2p.tile([128, ND], bf16, name="eF",
                                          tag="eF")
                            if gi == 2:
                                nc.scalar.activation(
                                    eF[:].rearrange("p (d n) -> p d n", n=N),
                                    e_raw[:, gi, None, :]
                                    .broadcast_to([128, D, N]), AF.Copy)
                            else:
                                nc.gpsimd.tensor_copy(
                                    eF[:].rearrange("p (d n) -> p d n", n=N),
                                    e_raw[:, gi, None, :]
                                    .broadcast_to([128, D, N]))
                            nc.vector.tensor_tensor(t2[:], u_sbs[gi][:],
                                                    eF[:], op=MUL)
                        for q in range(4):
                            nc.tensor.matmul(
                                s_ps[32 * q:32 * q + 32, :],
                                w4[:],
                                t2[:, q * 512:(q + 1) * 512],
                                start=(g == 0), stop=(g == NG - 1),
                                tile_position=(0, 32 * q),
                                skip_group_check=True,
                            )

                for sg in range(NSG):
                    u_sbs = u_phase(sg)
                    if state:
                        s_phase(*state.pop(0))
                    state.append((sg, u_sbs))
                while state:
                    s_phase(*state.pop(0))

                nc.vector.tensor_copy(s_evac[:], s_ps[:])
                nc.sync.dma_start(cc_in[:], s_evac[:])
                cc_and_squash(r)

            nc.sync.dma_start(v_d[:], v_sb[:])

    nc.compile()
    return nc


def prepare_inputs(x: np.ndarray, W: np.ndarray):
    """Full inputs -> per-core input maps (host-side reshuffles).
    All free dims are (d, n)-major."""
    import ml_dtypes
    bf = ml_dtypes.bfloat16

    ones4 = np.tile(np.eye(B, dtype=np.float32), (GRP, 1)).astype(bf)

    in_maps = []
    for k in range(NCORES):
        jlo, jhi = k * JC, (k + 1) * JC
        Wc = W[:, jlo:jhi]                       # [N, JC, D, I]
        xc = x[:, jlo:jhi]                       # [B, JC, I]
        arrw = np.ascontiguousarray(
            Wc.transpose(1, 3, 2, 0)).reshape(JC, I, ND)   # [j, i, (d,n)]
        arrx = np.ascontiguousarray(xc.transpose(1, 2, 0))  # [j, i, b]

        wa = arrw.reshape(NCH, 8, I, ND).transpose(1, 2, 0, 3) \
            .reshape(128, NCH, ND)
        xa = (arrx / N).reshape(NCH, 8, I, B).transpose(1, 2, 0, 3) \
            .reshape(128, NCH, B)

        wb4 = arrw.reshape(NSG, SG, GRP, I, ND)   # [sg, gi, a, i, nd]
        wbp = np.zeros((NSG, GRP, 32, SG, ND), dtype=np.float32)
        wbp[:, :, :I] = wb4.transpose(0, 2, 3, 1, 4)  # [sg, a, i, gi, nd]
        wb = wbp.reshape(NSG, 128, SG * ND)

        xb4 = arrx.reshape(NG, GRP, I, B)         # [g, a, i, b]
        xbp = np.zeros((GRP, 32, NG, B), dtype=np.float32)
        xbp[:, :I] = xb4.transpose(1, 2, 0, 3)    # [a, i, g, b]
        xb = xbp.reshape(128, NG, B)

        in_maps.append({
            "xa": np.ascontiguousarray(xa).astype(bf),
            "wa": np.ascontiguousarray(wa).astype(bf),
            "xb": np.ascontiguousarray(xb).astype(bf),
            "wb": np.ascontiguousarray(wb).astype(bf),
            "ones4": ones4,
        })
    return in_maps


def kernel(x: np.ndarray, W: np.ndarray) -> np.ndarray:
    from concourse.bass_utils import run_bass_kernel_spmd

    nc = _build()
    in_maps = prepare_inputs(x, W)
    res = run_bass_kernel_spmd(nc, in_maps, list(range(NCORES)))
    v = np.asarray(res.results[0]["v"], dtype=np.float32)
    # device layout is [B, (d, n)] -> [B, N, D]
    return np.ascontiguousarray(
        v.reshape(B, D, N).transpose(0, 2, 1))


if __name__ == "__main__":
    rng = np.random.default_rng(0)
    x = rng.normal(size=(B, J, I)).astype(np.float32)
    W = rng.normal(size=(N, J, D, I)).astype(np.float32) * 0.05
    v = kernel(x, W)
    print(v.shape, v.dtype, np.abs(v).max())


# revision 33
# speedup vs baseline: 1.7676x; 1.4546x over previous
"""CapsuleLayer dynamic-routing kernel for 8 Trainium2 NeuronCores.

Math (reference):
    u_hat[b,n,j,d] = sum_i W[n,j,d,i] * x[b,j,i]
    b = 0; for r in 0..2:
        c = softmax_n(b); s[b,n,d] = sum_j c*u_hat; v = squash_d(s)
        if r < 2: b += sum_d v*u_hat
    return v  [B, N, D]

Sharding: J (input capsules, 2048) split 8 ways -> Jc=256 per core.
Softmax over n is local; only s needs a 256 KiB AllReduce per iteration.

Per-core design (v3) — everything is (d, n)-major in the free dim:
  r0: c uniform -> s0 = (1/N) sum_{j,i} x*W, via K=128 matmuls over
      (j8,i) chunks; W layout-A [(j8,i), (d,n)] spans 128 partitions.
  r1/r2: j groups of 4 on DIAGONAL PE tiles (row band 32a = j=4g+a,
      operands at partitions 32a..32a+16; W tiles span 128 partitions,
      2 MiB DMAs). Per group:
        u in PSUM -> scalar evac bf16 [128, (d,n)]
        tl = u * v_rep           (DVE TT, dense, 2x)
        logits = sum_d tl        (5-level dense TT pyramid over outer d)
      Per supergroup of 4 groups: e_raw = exp(logits) batched; per
      group zsum/recip/log -> e2 = exp(logits - lnZ) on scalar (bias is
      per-partition), so t2 = e2 * u with an OUTER-dim broadcast of e2
      (dense inner runs -> fast on DVE/GpSimd; no stride-0 inner dims).
      s accumulated over all groups in one PSUM bank via ones4 matmuls.
  AllReduce s in fp32 via DRAM bounce; squash on the gathered [B,(d,n)]
  form (d-sum = dense pyramid); output transposed to [B,N,D] on host.
"""

import functools
import numpy as np

B, J, I = 32, 2048, 16
N, D = 64, 32
NCORES = 8
JC = J // NCORES          # 256 j per core
GRP = 4                   # j's per group (PE diagonal bands)
NG = JC // GRP            # 64 groups
SG = 4                    # groups per supergroup
NSG = NG // SG            # 16 supergroups
NCH = JC // 8             # 32 K=128 chunks for r0
ND = N * D                # 2048
HALF = ND // 2            # 1024
ROUTINGS = 3
EPS = 1e-7

GS_T2 = 3                 # how many of each supergroup's 4 t2 ops go to gpsimd


@functools.lru_cache(maxsize=1)
def _build():
    import concourse.bass as bass
    import concourse.mybir as mybir
    import concourse.bacc as bacc
    import concourse.tile as tile

    f32 = mybir.dt.float32
    bf16 = mybir.dt.bfloat16
    MUL = mybir.AluOpType.mult
    ADD = mybir.AluOpType.add
    AX = mybir.AxisListType.X
    AF = mybir.ActivationFunctionType

    nc = bacc.Bacc("TRN2", target_bir_lowering=False, debug=False,
                   num_devices=NCORES)

    xa_d = nc.dram_tensor("xa", [128, NCH, B], bf16, kind="ExternalInput")
    wa_d = nc.dram_tensor("wa", [128, NCH, ND], bf16, kind="ExternalInput")
    xb_d = nc.dram_tensor("xb", [128, NG, B], bf16, kind="ExternalInput")
    wb_d = nc.dram_tensor("wb", [NSG, 128, SG * ND], bf16, kind="ExternalInput")
    ones_d = nc.dram_tensor("ones4", [128, B], bf16, kind="ExternalInput")
    v_d = nc.dram_tensor("v", [B, ND], f32, kind="ExternalOutput")

    with tile.TileContext(nc) as tc:
        with (
            tc.tile_pool(name="persist", bufs=1) as pp,
            tc.tile_pool(name="wstream", bufs=2) as wp,
            tc.tile_pool(name="usb", bufs=10) as up,
            tc.tile_pool(name="tl", bufs=3) as tp,
            tc.tile_pool(name="pyr", bufs=2) as pyp,
            tc.tile_pool(name="t2", bufs=2) as t2p,
            tc.tile_pool(name="small", bufs=1) as sm,
            tc.tile_pool(name="soft", bufs=4) as sfp,
            tc.tile_pool(name="ups", bufs=3, space="PSUM") as ups_pool,
            tc.tile_pool(name="sps", bufs=1, space="PSUM") as sps_pool,
            tc.tile_pool(name="dram", bufs=1, space="DRAM") as dr,
        ):
            xa = pp.tile([128, NCH, B], bf16)
            nc.sync.dma_start(xa[:], xa_d[:])
            xb = pp.tile([128, NG, B], bf16)
            nc.sync.dma_start(xb[:], xb_d[:])
            ones4 = pp.tile([128, B], bf16)
            nc.sync.dma_start(ones4[:], ones_d[:])

            logits = pp.tile([128, NG, N], bf16)
            v_rep = pp.tile([128, ND], bf16)
            v_small = pp.tile([B, ND], bf16)
            s_full = pp.tile([B, ND], f32)
            v_sb = pp.tile([B, ND], f32)
            s0 = pp.tile([B, ND], f32)
            s_evac = pp.tile([128, 512], f32)
            sqt = pp.tile([B, ND], f32)
            sclFt = pp.tile([B, ND], f32)

            cc_in = dr.tile([128, 512], f32)
            cc_out = dr.tile([128, 512], f32)

            def cc_and_squash(r):
                """AllReduce cc_in -> cc_out, gather to [B,(d,n)], squash
                with a dense outer-d pyramid, write v_rep (r<2) / v_sb."""
                nc.gpsimd.collective_compute(
                    "AllReduce", ADD,
                    replica_groups=[list(range(NCORES))],
                    ins=[cc_in[:].opt()], outs=[cc_out[:].opt()],
                )
                for q in range(4):
                    nc.sync.dma_start(
                        s_full[:, q * 512:(q + 1) * 512],
                        cc_out[32 * q:32 * q + 32, :])

                s3 = s_full[:].rearrange("p (d n) -> p d n", n=N)
                sq = sqt[:].rearrange("p (d n) -> p d n", n=N)
                nc.vector.tensor_tensor(sq, s3, s3, op=MUL)
                h = D
                cur = sq
                while h > 1:
                    h //= 2
                    nxt = sm.tile([B, h, N], f32, name=f"sp{h}", tag=f"sp{h}")
                    nc.vector.tensor_tensor(
                        nxt[:], cur[:, 0:h, :], cur[:, h:2 * h, :], op=ADD)
                    cur = nxt[:]
                ns2f = cur.rearrange("p a n -> p (a n)")
                onep = sm.tile([B, N], f32, name="onep", tag="onep")
                nc.vector.tensor_scalar_add(onep[:], ns2f, 1.0)
                rt = sm.tile([B, N], f32, name="rt", tag="rt")
                eps_t = sm.tile([B, 1], f32, name="eps", tag="eps")
                nc.vector.memset(eps_t[:], EPS)
                nc.scalar.activation(rt[:], ns2f, AF.Sqrt, bias=eps_t[:])
                den = sm.tile([B, N], f32, name="den", tag="den")
                nc.vector.tensor_tensor(den[:], onep[:], rt[:], op=MUL)
                dinv = sm.tile([B, N], f32, name="dinv", tag="dinv")
                nc.vector.reciprocal(dinv[:], den[:])
                scl = sm.tile([B, N], f32, name="scl", tag="scl")
                nc.vector.tensor_tensor(scl[:], ns2f, dinv[:], op=MUL)
                sclF = sclFt[:].rearrange("p (d n) -> p d n", n=N)
                nc.scalar.activation(
                    sclF, scl[:, None, :].broadcast_to([B, D, N]),
                    AF.Copy)

                if r < ROUTINGS - 1:
                    nc.vector.tensor_tensor(
                        v_small[:].rearrange("p (d n) -> p d n", n=N), s3,
                        sclF, op=MUL)
                    for rr in range(GRP):
                        nc.sync.dma_start(
                            v_rep[32 * rr:32 * rr + 32, :], v_small[:])
                else:
                    nc.vector.tensor_tensor(
                        v_sb[:].rearrange("p (d n) -> p d n", n=N), s3,
                        sclF, op=MUL)

            # ---------------- r0: uniform c ----------------
            acc = [ups_pool.tile([128, HALF], f32, name="u_ps", tag="ups")
                   for _h in range(2)]
            CHB = 4
            for cb in range(NCH // CHB):
                wt = wp.tile([128, CHB * ND], bf16, name="w_t", tag="wst")
                nc.sync.dma_start(
                    wt[:], wa_d[:, cb * CHB:(cb + 1) * CHB, :])
                wt = wt[:].rearrange("p (c f) -> p c f", c=CHB)
                for cc_ in range(CHB):
                    ch = cb * CHB + cc_
                    for h in range(2):
                        for q in range(2):
                            nc.tensor.matmul(
                                acc[h][0:B, q * 512:(q + 1) * 512],
                                xa[:, ch, :],
                                wt[:, cc_, h * HALF + q * 512:
                                    h * HALF + (q + 1) * 512],
                                start=(ch == 0), stop=(ch == NCH - 1),
                                skip_group_check=True,
                            )
            for h in range(2):
                nc.scalar.activation(
                    s0[:, h * HALF:(h + 1) * HALF], acc[h][0:B, :], AF.Copy)
            for q in range(4):
                nc.sync.dma_start(
                    cc_in[32 * q:32 * q + 32, :],
                    s0[:, q * 512:(q + 1) * 512])
            cc_and_squash(0)

            # ---------------- r1, r2 ----------------
            for r in range(1, ROUTINGS):
                s_ps = sps_pool.tile([128, 512], f32)

                state = []

                def u_phase(sg):
                    wt = wp.tile([128, SG * ND], bf16, name="w_t", tag="wst")
                    nc.sync.dma_start(wt[:], wb_d[sg, :, :])
                    wtv = wt[:].rearrange("p (g f) -> p g f", g=SG)
                    u_sbs = []
                    sgP = pyp.tile([128, SG, 8, N], bf16, name="sgP",
                                   tag="sgP")
                    for gi in range(SG):
                        g = sg * SG + gi
                        u_sb = up.tile([128, ND], bf16, name="u_sb", tag="usb")
                        for h in range(2):
                            u_ps = ups_pool.tile([128, HALF], f32,
                                                 name="u_ps", tag="ups")
                            for a in range(GRP):
                                for q in range(2):
                                    nc.tensor.matmul(
                                        u_ps[32 * a:32 * a + 32,
                                             q * 512:(q + 1) * 512],
                                        xb[32 * a:32 * a + 16, g, :],
                                        wtv[32 * a:32 * a + 16, gi,
                                            h * HALF + q * 512:
                                            h * HALF + (q + 1) * 512],
                                        start=True, stop=True,
                                        tile_position=(32 * a, 32 * a),
                                        skip_group_check=True,
                                    )
                            nc.scalar.activation(
                                u_sb[:, h * HALF:(h + 1) * HALF],
                                u_ps[:], AF.Copy)
                        # tl = u * v_rep  (flat 2D, bf16, 2x mode)
                        tl = tp.tile([128, ND], bf16, name="tl", tag="tl")
                        nc.vector.tensor_tensor(tl[:], u_sb[:], v_rep[:],
                                                op=MUL)
                        # sum over outer d: L1+L2 per group, tail batched
                        with nc.allow_low_precision("bf16 logits pyramid"):
                            tl3 = tl[:].rearrange("p (d n) -> p d n", n=N)
                            p16 = pyp.tile([128, 16, N], bf16, name="p16",
                                           tag="p16")
                            nc.vector.tensor_tensor(
                                p16[:], tl3[:, 0:16, :], tl3[:, 16:32, :],
                                op=ADD)
                            nc.vector.tensor_tensor(
                                sgP[:, gi], p16[:, 0:8, :], p16[:, 8:16, :],
                                op=ADD)
                        u_sbs.append(u_sb)
                    # batched pyramid tail over the supergroup
                    sl = slice(sg * SG, (sg + 1) * SG)
                    with nc.allow_low_precision("bf16 logits pyramid"):
                        t4 = pyp.tile([128, SG, 4, N], bf16, name="t4",
                                      tag="t4")
                        nc.vector.tensor_tensor(
                            t4[:], sgP[:, :, 0:4, :], sgP[:, :, 4:8, :],
                            op=ADD)
                        t2_ = pyp.tile([128, SG, 2, N], bf16, name="t2_",
                                       tag="t2_")
                        nc.vector.tensor_tensor(
                            t2_[:], t4[:, :, 0:2, :], t4[:, :, 2:4, :],
                            op=ADD)
                        if r == 1:
                            nc.vector.tensor_tensor(
                                logits[:, sl, :], t2_[:, :, 0, :],
                                t2_[:, :, 1, :], op=ADD)
                        else:
                            dsg = pyp.tile([128, SG, N], bf16, name="dsg",
                                           tag="dsg")
                            nc.vector.tensor_tensor(
                                dsg[:], t2_[:, :, 0, :], t2_[:, :, 1, :],
                                op=ADD)
                            nc.vector.tensor_add(
                                logits[:, sl, :], logits[:, sl, :], dsg[:])
                    return u_sbs

                def s_phase(sg, u_sbs):
                    sl = slice(sg * SG, (sg + 1) * SG)
                    e_raw = sfp.tile([128, SG, N], bf16, name="eraw",
                                     tag="eraw")
                    nc.scalar.activation(e_raw[:], logits[:, sl, :], AF.Exp)
                    for gi in range(SG):
                        g = sg * SG + gi
                        zsum = sfp.tile([128, 1], f32, name="zs", tag="zs")
                        nc.vector.tensor_reduce(
                            zsum[:], e_raw[:, gi, :], axis=AX, op=ADD)
                        zrec = sfp.tile([128, 1], f32, name="zr", tag="zr")
                        nc.vector.reciprocal(zrec[:], zsum[:])
                        # w4 = ones4 * zinv (dense; zinv expanded on scalar)
                        zinvB = sfp.tile([128, B], bf16, name="zb", tag="zb")
                        nc.scalar.activation(
                            zinvB[:], zrec[:].broadcast_to([128, B]),
                            AF.Copy)
                        w4 = sfp.tile([128, B], bf16, name="w4", tag="w4")
                        nc.vector.tensor_tensor(w4[:], ones4[:], zinvB[:],
                                                op=MUL)
                        t2 = t2p.tile([128, ND], bf16, name="t2", tag="t2")
                        # dense e via scalar exp with broadcast src, then
                        # dense DVE multiply (gpsimd SBUF traffic blocks
                        # the DVE shared read port — keep it off the path)
                        eF = t2p.tile([128, ND], bf16, name="eF", tag="eF")
                        nc.scalar.activation(
                            eF[:].rearrange("p (d n) -> p d n", n=N),
                            logits[:, g, None, :]
                            .broadcast_to([128, D, N]), AF.Exp)
                        nc.vector.tensor_tensor(t2[:], u_sbs[gi][:],
                                                eF[:], op=MUL)
                        for q in range(4):
                            nc.tensor.matmul(
                                s_ps[32 * q:32 * q + 32, :],
                                w4[:],
                                t2[:, q * 512:(q + 1) * 512],
                                start=(g == 0), stop=(g == NG - 1),
                                tile_position=(0, 32 * q),
                                skip_group_check=True,
                            )

                for sg in range(NSG):
                    u_sbs = u_phase(sg)
                    if state:
                        s_phase(*state.pop(0))
                    state.append((sg, u_sbs))
                while state:
                    s_phase(*state.pop(0))

                nc.vector.tensor_copy(s_evac[:], s_ps[:])
                nc.sync.dma_start(cc_in[:], s_evac[:])
                cc_and_squash(r)

            nc.sync.dma_start(v_d[:], v_sb[:])

    nc.compile()
    return nc


def prepare_inputs(x: np.ndarray, W: np.ndarray):
    """Full inputs -> per-core input maps (host-side reshuffles).
    All free dims are (d, n)-major."""
    import ml_dtypes
    bf = ml_dtypes.bfloat16

    ones4 = np.tile(np.eye(B, dtype=np.float32), (GRP, 1)).astype(bf)

    in_maps = []
    for k in range(NCORES):
        jlo, jhi = k * JC, (k + 1) * JC
        Wc = W[:, jlo:jhi]                       # [N, JC, D, I]
        xc = x[:, jlo:jhi]                       # [B, JC, I]
        arrw = np.ascontiguousarray(
            Wc.transpose(1, 3, 2, 0)).reshape(JC, I, ND)   # [j, i, (d,n)]
        arrx = np.ascontiguousarray(xc.transpose(1, 2, 0))  # [j, i, b]

        wa = arrw.reshape(NCH, 8, I, ND).transpose(1, 2, 0, 3) \
            .reshape(128, NCH, ND)
        xa = (arrx / N).reshape(NCH, 8, I, B).transpose(1, 2, 0, 3) \
            .reshape(128, NCH, B)

        wb4 = arrw.reshape(NSG, SG, GRP, I, ND)   # [sg, gi, a, i, nd]
        wbp = np.zeros((NSG, GRP, 32, SG, ND), dtype=np.float32)
        wbp[:, :, :I] = wb4.transpose(0, 2, 3, 1, 4)  # [sg, a, i, gi, nd]
        wb = wbp.reshape(NSG, 128, SG * ND)

        xb4 = arrx.reshape(NG, GRP, I, B)         # [g, a, i, b]
        xbp = np.zeros((GRP, 32, NG, B), dtype=np.float32)
        xbp[:, :I] = xb4.transpose(1, 2, 0, 3)    # [a, i, g, b]
        xb = xbp.reshape(128, NG, B)

        in_maps.append({
            "xa": np.ascontiguousarray(xa).astype(bf),
            "wa": np.ascontiguousarray(wa).astype(bf),
            "xb": np.ascontiguousarray(xb).astype(bf),
            "wb": np.ascontiguousarray(wb).astype(bf),
            "ones4": ones4,
        })
    return in_maps


def kernel(x: np.ndarray, W: np.ndarray) -> np.ndarray:
    from concourse.bass_utils import run_bass_kernel_spmd

    nc = _build()
    in_maps = prepare_inputs(x, W)
    res = run_bass_kernel_spmd(nc, in_maps, list(range(NCORES)))
    v = np.asarray(res.results[0]["v"], dtype=np.float32)
    # device layout is [B, (d, n)] -> [B, N, D]
    return np.ascontiguousarray(
        v.reshape(B, D, N).transpose(0, 2, 1))


if __name__ == "__main__":
    rng = np.random.default_rng(0)
    x = rng.normal(size=(B, J, I)).astype(np.float32)
    W = rng.normal(size=(N, J, D, I)).astype(np.float32) * 0.05
    v = kernel(x, W)
    print(v.shape, v.dtype, np.abs(v).max())


# revision 36
# speedup vs baseline: 1.7773x; 1.0054x over previous
"""CapsuleLayer dynamic-routing kernel for 8 Trainium2 NeuronCores.

Math (reference):
    u_hat[b,n,j,d] = sum_i W[n,j,d,i] * x[b,j,i]
    b = 0; for r in 0..2:
        c = softmax_n(b); s[b,n,d] = sum_j c*u_hat; v = squash_d(s)
        if r < 2: b += sum_d v*u_hat
    return v  [B, N, D]

Sharding: J (input capsules, 2048) split 8 ways -> Jc=256 per core.
Softmax over n is local; only s needs a 256 KiB AllReduce per iteration.

Per-core design (v3) — everything is (d, n)-major in the free dim:
  r0: c uniform -> s0 = (1/N) sum_{j,i} x*W, via K=128 matmuls over
      (j8,i) chunks; W layout-A [(j8,i), (d,n)] spans 128 partitions.
  r1/r2: j groups of 4 on DIAGONAL PE tiles (row band 32a = j=4g+a,
      operands at partitions 32a..32a+16; W tiles span 128 partitions,
      2 MiB DMAs). Per group:
        u in PSUM -> scalar evac bf16 [128, (d,n)]
        tl = u * v_rep           (DVE TT, dense, 2x)
        logits = sum_d tl        (5-level dense TT pyramid over outer d)
      Per supergroup of 4 groups: e_raw = exp(logits) batched; per
      group zsum/recip/log -> e2 = exp(logits - lnZ) on scalar (bias is
      per-partition), so t2 = e2 * u with an OUTER-dim broadcast of e2
      (dense inner runs -> fast on DVE/GpSimd; no stride-0 inner dims).
      s accumulated over all groups in one PSUM bank via ones4 matmuls.
  AllReduce s in fp32 via DRAM bounce; squash on the gathered [B,(d,n)]
  form (d-sum = dense pyramid); output transposed to [B,N,D] on host.
"""

import functools
import numpy as np

B, J, I = 32, 2048, 16
N, D = 64, 32
NCORES = 8
JC = J // NCORES          # 256 j per core
GRP = 4                   # j's per group (PE diagonal bands)
NG = JC // GRP            # 64 groups
SG = 4                    # groups per supergroup
NSG = NG // SG            # 16 supergroups
NCH = JC // 8             # 32 K=128 chunks for r0
ND = N * D                # 2048
HALF = ND // 2            # 1024
ROUTINGS = 3
EPS = 1e-7

GS_T2 = 3                 # how many of each supergroup's 4 t2 ops go to gpsimd


@functools.lru_cache(maxsize=1)
def _build():
    import concourse.bass as bass
    import concourse.mybir as mybir
    import concourse.bacc as bacc
    import concourse.tile as tile

    f32 = mybir.dt.float32
    bf16 = mybir.dt.bfloat16
    MUL = mybir.AluOpType.mult
    ADD = mybir.AluOpType.add
    AX = mybir.AxisListType.X
    AF = mybir.ActivationFunctionType

    nc = bacc.Bacc("TRN2", target_bir_lowering=False, debug=False,
                   num_devices=NCORES)

    xa_d = nc.dram_tensor("xa", [128, NCH, B], bf16, kind="ExternalInput")
    wa_d = nc.dram_tensor("wa", [128, NCH, ND], bf16, kind="ExternalInput")
    xb_d = nc.dram_tensor("xb", [128, NG, B], bf16, kind="ExternalInput")
    wb_d = nc.dram_tensor("wb", [NSG, 128, SG * ND], bf16, kind="ExternalInput")
    ones_d = nc.dram_tensor("ones4", [128, B], bf16, kind="ExternalInput")
    v_d = nc.dram_tensor("v", [B, ND], f32, kind="ExternalOutput")

    with tile.TileContext(nc) as tc:
        with (
            tc.tile_pool(name="persist", bufs=1) as pp,
            tc.tile_pool(name="wstream", bufs=2) as wp,
            tc.tile_pool(name="usb", bufs=10) as up,
            tc.tile_pool(name="tl", bufs=3) as tp,
            tc.tile_pool(name="pyr", bufs=2) as pyp,
            tc.tile_pool(name="t2", bufs=2) as t2p,
            tc.tile_pool(name="small", bufs=1) as sm,
            tc.tile_pool(name="soft", bufs=4) as sfp,
            tc.tile_pool(name="ups", bufs=3, space="PSUM") as ups_pool,
            tc.tile_pool(name="sps", bufs=1, space="PSUM") as sps_pool,
            tc.tile_pool(name="dram", bufs=1, space="DRAM") as dr,
        ):
            xa = pp.tile([128, NCH, B], bf16)
            nc.sync.dma_start(xa[:], xa_d[:])
            xb = pp.tile([128, NG, B], bf16)
            nc.sync.dma_start(xb[:], xb_d[:])
            ones4 = pp.tile([128, B], bf16)
            nc.sync.dma_start(ones4[:], ones_d[:])

            logits = pp.tile([128, NG, N], bf16)
            v_rep = pp.tile([128, ND], bf16)
            v_small = pp.tile([B, ND], bf16)
            s_full = pp.tile([B, ND], f32)
            v_sb = pp.tile([B, ND], f32)
            s0 = pp.tile([B, ND], f32)
            s_evac = pp.tile([128, 512], f32)
            sqt = pp.tile([B, ND], f32)
            sclFt = pp.tile([B, ND], f32)

            cc_in = dr.tile([128, 512], f32)
            cc_out = dr.tile([128, 512], f32)

            def cc_and_squash(r):
                """AllReduce cc_in -> cc_out, gather to [B,(d,n)], squash
                with a dense outer-d pyramid, write v_rep (r<2) / v_sb."""
                nc.gpsimd.collective_compute(
                    "AllReduce", ADD,
                    replica_groups=[list(range(NCORES))],
                    ins=[cc_in[:].opt()], outs=[cc_out[:].opt()],
                )
                for q in range(4):
                    nc.sync.dma_start(
                        s_full[:, q * 512:(q + 1) * 512],
                        cc_out[32 * q:32 * q + 32, :])

                s3 = s_full[:].rearrange("p (d n) -> p d n", n=N)
                sq = sqt[:].rearrange("p (d n) -> p d n", n=N)
                nc.vector.tensor_tensor(sq, s3, s3, op=MUL)
                h = D
                cur = sq
                while h > 1:
                    h //= 2
                    nxt = sm.tile([B, h, N], f32, name=f"sp{h}", tag=f"sp{h}")
                    nc.vector.tensor_tensor(
                        nxt[:], cur[:, 0:h, :], cur[:, h:2 * h, :], op=ADD)
                    cur = nxt[:]
                ns2f = cur.rearrange("p a n -> p (a n)")
                onep = sm.tile([B, N], f32, name="onep", tag="onep")
                nc.vector.tensor_scalar_add(onep[:], ns2f, 1.0)
                rt = sm.tile([B, N], f32, name="rt", tag="rt")
                eps_t = sm.tile([B, 1], f32, name="eps", tag="eps")
                nc.vector.memset(eps_t[:], EPS)
                nc.scalar.activation(rt[:], ns2f, AF.Sqrt, bias=eps_t[:])
                den = sm.tile([B, N], f32, name="den", tag="den")
                nc.vector.tensor_tensor(den[:], onep[:], rt[:], op=MUL)
                dinv = sm.tile([B, N], f32, name="dinv", tag="dinv")
                nc.vector.reciprocal(dinv[:], den[:])
                scl = sm.tile([B, N], f32, name="scl", tag="scl")
                nc.vector.tensor_tensor(scl[:], ns2f, dinv[:], op=MUL)
                sclF = sclFt[:].rearrange("p (d n) -> p d n", n=N)
                nc.scalar.activation(
                    sclF, scl[:, None, :].broadcast_to([B, D, N]),
                    AF.Copy)

                if r < ROUTINGS - 1:
                    nc.vector.tensor_tensor(
                        v_small[:].rearrange("p (d n) -> p d n", n=N), s3,
                        sclF, op=MUL)
                    for rr in range(GRP):
                        nc.sync.dma_start(
                            v_rep[32 * rr:32 * rr + 32, :], v_small[:])
                else:
                    nc.vector.tensor_tensor(
                        v_sb[:].rearrange("p (d n) -> p d n", n=N), s3,
                        sclF, op=MUL)

            # ---------------- r0: uniform c ----------------
            acc = [ups_pool.tile([128, HALF], f32, name="u_ps", tag="ups")
                   for _h in range(2)]
            CHB = 4
            for cb in range(NCH // CHB):
                wt = wp.tile([128, CHB * ND], bf16, name="w_t", tag="wst")
                nc.sync.dma_start(
                    wt[:], wa_d[:, cb * CHB:(cb + 1) * CHB, :])
                wt = wt[:].rearrange("p (c f) -> p c f", c=CHB)
                for cc_ in range(CHB):
                    ch = cb * CHB + cc_
                    for h in range(2):
                        for q in range(2):
                            nc.tensor.matmul(
                                acc[h][0:B, q * 512:(q + 1) * 512],
                                xa[:, ch, :],
                                wt[:, cc_, h * HALF + q * 512:
                                    h * HALF + (q + 1) * 512],
                                start=(ch == 0), stop=(ch == NCH - 1),
                                skip_group_check=True,
                            )
            for h in range(2):
                nc.scalar.activation(
                    s0[:, h * HALF:(h + 1) * HALF], acc[h][0:B, :], AF.Copy)
            for q in range(4):
                nc.sync.dma_start(
                    cc_in[32 * q:32 * q + 32, :],
                    s0[:, q * 512:(q + 1) * 512])
            pending_tail = [lambda: cc_and_squash(0)]

            # ---------------- r1, r2 ----------------
            for r in range(1, ROUTINGS):
                s_ps = sps_pool.tile([128, 512], f32)

                state = []

                def u_mm_phase(sg):
                    wt = wp.tile([128, SG * ND], bf16, name="w_t", tag="wst")
                    nc.sync.dma_start(wt[:], wb_d[sg, :, :])
                    wtv = wt[:].rearrange("p (g f) -> p g f", g=SG)
                    u_sbs = []
                    for gi in range(SG):
                        g = sg * SG + gi
                        u_sb = up.tile([128, ND], bf16, name="u_sb", tag="usb")
                        for h in range(2):
                            u_ps = ups_pool.tile([128, HALF], f32,
                                                 name="u_ps", tag="ups")
                            for a in range(GRP):
                                for q in range(2):
                                    nc.tensor.matmul(
                                        u_ps[32 * a:32 * a + 32,
                                             q * 512:(q + 1) * 512],
                                        xb[32 * a:32 * a + 16, g, :],
                                        wtv[32 * a:32 * a + 16, gi,
                                            h * HALF + q * 512:
                                            h * HALF + (q + 1) * 512],
                                        start=True, stop=True,
                                        tile_position=(32 * a, 32 * a),
                                        skip_group_check=True,
                                    )
                            nc.scalar.activation(
                                u_sb[:, h * HALF:(h + 1) * HALF],
                                u_ps[:], AF.Copy)
                        u_sbs.append(u_sb)
                    return u_sbs

                def tl_phase(sg, u_sbs):
                    sgP = pyp.tile([128, SG, 8, N], bf16, name="sgP",
                                   tag="sgP")
                    for gi in range(SG):
                        g = sg * SG + gi
                        u_sb = u_sbs[gi]
                        # tl = u * v_rep  (flat 2D, bf16, 2x mode)
                        tl = tp.tile([128, ND], bf16, name="tl", tag="tl")
                        nc.vector.tensor_tensor(tl[:], u_sb[:], v_rep[:],
                                                op=MUL)
                        # sum over outer d: L1+L2 per group, tail batched
                        with nc.allow_low_precision("bf16 logits pyramid"):
                            tl3 = tl[:].rearrange("p (d n) -> p d n", n=N)
                            p16 = pyp.tile([128, 16, N], bf16, name="p16",
                                           tag="p16")
                            nc.vector.tensor_tensor(
                                p16[:], tl3[:, 0:16, :], tl3[:, 16:32, :],
                                op=ADD)
                            nc.vector.tensor_tensor(
                                sgP[:, gi], p16[:, 0:8, :], p16[:, 8:16, :],
                                op=ADD)
                    # batched pyramid tail over the supergroup
                    sl = slice(sg * SG, (sg + 1) * SG)
                    with nc.allow_low_precision("bf16 logits pyramid"):
                        t4 = pyp.tile([128, SG, 4, N], bf16, name="t4",
                                      tag="t4")
                        nc.vector.tensor_tensor(
                            t4[:], sgP[:, :, 0:4, :], sgP[:, :, 4:8, :],
                            op=ADD)
                        t2_ = pyp.tile([128, SG, 2, N], bf16, name="t2_",
                                       tag="t2_")
                        nc.vector.tensor_tensor(
                            t2_[:], t4[:, :, 0:2, :], t4[:, :, 2:4, :],
                            op=ADD)
                        if r == 1:
                            nc.vector.tensor_tensor(
                                logits[:, sl, :], t2_[:, :, 0, :],
                                t2_[:, :, 1, :], op=ADD)
                        else:
                            dsg = pyp.tile([128, SG, N], bf16, name="dsg",
                                           tag="dsg")
                            nc.vector.tensor_tensor(
                                dsg[:], t2_[:, :, 0, :], t2_[:, :, 1, :],
                                op=ADD)
                            nc.vector.tensor_add(
                                logits[:, sl, :], logits[:, sl, :], dsg[:])

                def s_phase(sg, u_sbs):
                    sl = slice(sg * SG, (sg + 1) * SG)
                    e_raw = sfp.tile([128, SG, N], bf16, name="eraw",
                                     tag="eraw")
                    nc.scalar.activation(e_raw[:], logits[:, sl, :], AF.Exp)
                    for gi in range(SG):
                        g = sg * SG + gi
                        zsum = sfp.tile([128, 1], f32, name="zs", tag="zs")
                        nc.vector.tensor_reduce(
                            zsum[:], e_raw[:, gi, :], axis=AX, op=ADD)
                        zrec = sfp.tile([128, 1], f32, name="zr", tag="zr")
                        nc.vector.reciprocal(zrec[:], zsum[:])
                        # w4 = ones4 * zinv (dense; zinv expanded on scalar)
                        zinvB = sfp.tile([128, B], bf16, name="zb", tag="zb")
                        nc.scalar.activation(
                            zinvB[:], zrec[:].broadcast_to([128, B]),
                            AF.Copy)
                        w4 = sfp.tile([128, B], bf16, name="w4", tag="w4")
                        nc.vector.tensor_tensor(w4[:], ones4[:], zinvB[:],
                                                op=MUL)
                        t2 = t2p.tile([128, ND], bf16, name="t2", tag="t2")
                        # dense e via scalar exp with broadcast src, then
                        # dense DVE multiply (gpsimd SBUF traffic blocks
                        # the DVE shared read port — keep it off the path)
                        eF = t2p.tile([128, ND], bf16, name="eF", tag="eF")
                        nc.scalar.activation(
                            eF[:].rearrange("p (d n) -> p d n", n=N),
                            logits[:, g, None, :]
                            .broadcast_to([128, D, N]), AF.Exp)
                        nc.vector.tensor_tensor(t2[:], u_sbs[gi][:],
                                                eF[:], op=MUL)
                        for q in range(4):
                            nc.tensor.matmul(
                                s_ps[32 * q:32 * q + 32, :],
                                w4[:],
                                t2[:, q * 512:(q + 1) * 512],
                                start=(g == 0), stop=(g == NG - 1),
                                tile_position=(0, 32 * q),
                                skip_group_check=True,
                            )

                for sg in range(NSG):
                    u_sbs = u_mm_phase(sg)
                    if sg == 0 and pending_tail:
                        pending_tail.pop(0)()
                    tl_phase(sg, u_sbs)
                    if state:
                        s_phase(*state.pop(0))
                    state.append((sg, u_sbs))
                while state:
                    s_phase(*state.pop(0))

                nc.vector.tensor_copy(s_evac[:], s_ps[:])
                nc.sync.dma_start(cc_in[:], s_evac[:])
                pending_tail.append(lambda rr=r: cc_and_squash(rr))

            pending_tail.pop(0)()
            nc.sync.dma_start(v_d[:], v_sb[:])

    nc.compile()
    return nc


def prepare_inputs(x: np.ndarray, W: np.ndarray):
    """Full inputs -> per-core input maps (host-side reshuffles).
    All free dims are (d, n)-major."""
    import ml_dtypes
    bf = ml_dtypes.bfloat16

    ones4 = np.tile(np.eye(B, dtype=np.float32), (GRP, 1)).astype(bf)

    in_maps = []
    for k in range(NCORES):
        jlo, jhi = k * JC, (k + 1) * JC
        Wc = W[:, jlo:jhi]                       # [N, JC, D, I]
        xc = x[:, jlo:jhi]                       # [B, JC, I]
        arrw = np.ascontiguousarray(
            Wc.transpose(1, 3, 2, 0)).reshape(JC, I, ND)   # [j, i, (d,n)]
        arrx = np.ascontiguousarray(xc.transpose(1, 2, 0))  # [j, i, b]

        wa = arrw.reshape(NCH, 8, I, ND).transpose(1, 2, 0, 3) \
            .reshape(128, NCH, ND)
        xa = (arrx / N).reshape(NCH, 8, I, B).transpose(1, 2, 0, 3) \
            .reshape(128, NCH, B)

        wb4 = arrw.reshape(NSG, SG, GRP, I, ND)   # [sg, gi, a, i, nd]
        wbp = np.zeros((NSG, GRP, 32, SG, ND), dtype=np.float32)
        wbp[:, :, :I] = wb4.transpose(0, 2, 3, 1, 4)  # [sg, a, i, gi, nd]
        wb = wbp.reshape(NSG, 128, SG * ND)

        xb4 = arrx.reshape(NG, GRP, I, B)         # [g, a, i, b]
        xbp = np.zeros((GRP, 32, NG, B), dtype=np.float32)
        xbp[:, :I] = xb4.transpose(1, 2, 0, 3)    # [a, i, g, b]
        xb = xbp.reshape(128, NG, B)

        in_maps.append({
            "xa": np.ascontiguousarray(xa).astype(bf),
            "wa": np.ascontiguousarray(wa).astype(bf),
            "xb": np.ascontiguousarray(xb).astype(bf),
            "wb": np.ascontiguousarray(wb).astype(bf),
            "ones4": ones4,
        })
    return in_maps


def kernel(x: np.ndarray, W: np.ndarray) -> np.ndarray:
    from concourse.bass_utils import run_bass_kernel_spmd

    nc = _build()
    in_maps = prepare_inputs(x, W)
    res = run_bass_kernel_spmd(nc, in_maps, list(range(NCORES)))
    v = np.asarray(res.results[0]["v"], dtype=np.float32)
    # device layout is [B, (d, n)] -> [B, N, D]
    return np.ascontiguousarray(
        v.reshape(B, D, N).transpose(0, 2, 1))


if __name__ == "__main__":
    rng = np.random.default_rng(0)
    x = rng.normal(size=(B, J, I)).astype(np.float32)
    W = rng.normal(size=(N, J, D, I)).astype(np.float32) * 0.05
    v = kernel(x, W)
    print(v.shape, v.dtype, np.abs(v).max())
